# revision 49
# baseline (speedup 1.0000x reference)
"""GQA attention (RoPE + causal softmax + out-proj) on 8 TRN2 cores.

Sharding: one core per (batch b, kv-head-group g): 2 batches x 4 kv groups = 8
cores. Each core computes its group's 4 query heads end to end, including the
partial output projection through its 256 rows of wo; the host sums the 4
fp16 partial projections per batch in fp32.

Per-core kernel layout (all "transposed domain": feature dims on partitions,
sequence on the free axis):
  qT [256, S] = wq_g^T x^T, computed as matmul(lhsT=wq_g, rhs=xT); wq columns
  are host-permuted so psum M-tile 0 holds all even (te) rope components
  (4 heads x 32) and M-tile 1 all odd (to). RoPE is then whole-tile vector
  ops against cos/sin tables replicated per head. Same for k (rows 0:64 of
  the packed kv projection; v = rows 64:128).
  scores_T [k 128, q W] per (head-pair, k-block, q-chunk): one row-packed
  concurrent matmul pair (K=64 each at tile_position (0,0)/(64,0)); for
  diagonal-band k-blocks the fully-masked leading q-subblocks are skipped
  (W = 512-128*jj). exp via ACT (scale=1/8); causality enforced by a
  triangular multiply on the 128-wide diagonal subblock of p after exp.
  PV: out_T [65, q W] accumulated over k-blocks: matmul(lhsT=v_aug[k,65],
  rhs=p[k,qW]); v_aug column 64 is ones, so row 64 accumulates the softmax
  denominator. Normalization: reciprocal_approx_fast of that row, broadcast
  to 128 partitions via a tiny K=2 matmul against a selection matrix.
  Out-proj: matmul(lhsT=attT[c, s-block], rhs=wo[c, e-chunk]) -> [2048,1024]
  fp16 partial, DMA'd out.
"""

import os
import sys
import types

import numpy as np


def _ensure_axon_hooks_shim():
    """The agent image's antenv package lacks the axon_hooks submodule that
    concourse's trace path imports; install a stub so trace requests degrade
    to no-trace instead of crashing (a real hook can be set into the stub)."""
    try:
        import antenv.axon_hooks  # noqa: F401

        return
    except ImportError:
        pass
    try:
        import antenv
    except ImportError:
        return
    mod = types.ModuleType("antenv.axon_hooks")
    mod._AXON_NTFF_PROFILE_HOOK = None

    def get_axon_ntff_profile_hook():
        return mod._AXON_NTFF_PROFILE_HOOK

    def set_axon_ntff_profile_hook(hook):
        mod._AXON_NTFF_PROFILE_HOOK = hook

    mod.get_axon_ntff_profile_hook = get_axon_ntff_profile_hook
    mod.set_axon_ntff_profile_hook = set_axon_ntff_profile_hook
    sys.modules["antenv.axon_hooks"] = mod
    antenv.axon_hooks = mod


_ensure_axon_hooks_shim()

import concourse.bass as bass
import concourse.bacc as bacc
import concourse.mybir as mybir
import concourse.tile as tile
from concourse.bass_utils import run_bass_kernel_spmd

F32 = mybir.dt.float32
F16 = mybir.dt.float16
I16 = mybir.dt.int16
AF = mybir.ActivationFunctionType
OP = mybir.AluOpType

# Schraudolph-style fp16 exp-by-bitcast constants: trunc(x*EXP_A + EXP_B)
# interpreted as fp16 bits approximates exp(0.125*x) with ~3% max element
# error that cancels in the softmax ratio (calibrated for DVE truncation).
EXP_A = 1477.3197 * 0.125  # 2^10/ln2 * score scale
EXP_B = 15316.0

B, DIM = 2, 1024
NH, NKV, HD = 16, 4, 64
GH = NH // NKV  # query heads per kv group = 4
S_FULL = 2048
SC = 512  # q chunk width


def build_nc(S=S_FULL, n_cores=8):
    NCH = S // SC
    NKB = S // 128
    KT = DIM // 128  # 8 k-tiles over the model dim

    nc = bacc.Bacc(
        "TRN2", target_bir_lowering=False, debug=False, num_devices=n_cores
    )
    xT = nc.dram_tensor("xT", [DIM, S], F16, kind="ExternalInput").ap()
    wq = nc.dram_tensor("wq", [DIM, 256], F16, kind="ExternalInput").ap()
    wkv = nc.dram_tensor("wkv", [DIM, 128], F16, kind="ExternalInput").ap()
    wo = nc.dram_tensor("wo", [256, DIM], F16, kind="ExternalInput").ap()
    cosr = nc.dram_tensor("cosr", [128, S], F16, kind="ExternalInput").ap()
    sinr = nc.dram_tensor("sinr", [128, S], F16, kind="ExternalInput").ap()
    tri2 = nc.dram_tensor("tri2", [128, 256], F16, kind="ExternalInput").ap()
    ident = nc.dram_tensor("ident", [128, 128], F16, kind="ExternalInput").ap()
    sel2 = nc.dram_tensor("sel2", [33, 128], F32, kind="ExternalInput").ap()
    out = nc.dram_tensor("out", [S, DIM], F16, kind="ExternalOutput").ap()

    xT3 = xT.rearrange("(k p) s -> k p s", p=128)

    with tile.TileContext(nc) as tc:
        with (
            tc.tile_pool(name="const", bufs=1) as cp,
            tc.tile_pool(name="qps", bufs=2, space="PSUM") as qps,
            tc.tile_pool(name="scps", bufs=2, space="PSUM") as bps,
            tc.tile_pool(name="otps", bufs=2, space="PSUM") as ops,
            tc.tile_pool(name="xt", bufs=16) as xp,
            tc.tile_pool(name="rt", bufs=3) as rt,
            tc.tile_pool(name="pp", bufs=6) as pp,
            tc.tile_pool(name="np_", bufs=2) as npo,
            tc.tile_pool(name="op", bufs=3) as op_pool,
        ):
            COS = cp.tile([128, S], F16, tag="COS")
            SIN = cp.tile([128, S], F16, tag="SIN")
            WQ = cp.tile([128, KT, 256], F16, tag="WQ")
            WKV = cp.tile([128, KT, 128], F16, tag="WKV")
            WO = cp.tile([128, 2, DIM], F16, tag="WO")
            TRI2 = cp.tile([128, 2, 128], F16, tag="TRI2")
            IDENT = cp.tile([128, 128], F16, tag="IDENT")
            SEL2 = cp.tile([33, 128], F32, tag="SEL2")
            KA4 = cp.tile([128, S], F16, tag="KA4")  # KAB: [KA;KB;KA;KB]
            REIM0 = cp.tile([128, S], F16, tag="REIM0")
            REIM1 = cp.tile([128, S], F16, tag="REIM1")
            VAUG = cp.tile([128, NKB, 128], F16, tag="VAUG")
            AT0 = cp.tile([128, S], F16, tag="AT0")
            AT1 = cp.tile([128, S], F16, tag="AT1")

            # weight/const loads, split across queues so the critical path
            # (WQ/WKV + first x chunk on the sync queue) isn't serialized
            # behind the rest.
            wq_r = wq.rearrange("(k p) m -> p k m", p=128)
            wkv_r = wkv.rearrange("(k p) m -> p k m", p=128)
            for kt in range(KT):
                eng = (nc.sync, nc.scalar, nc.gpsimd)[kt % 3]
                eng.dma_start(WQ[:, kt, :], wq_r[:, kt, :])
                eng2 = (nc.scalar, nc.gpsimd, nc.sync)[kt % 3]
                eng2.dma_start(WKV[:, kt, :], wkv_r[:, kt, :])
            nc.scalar.dma_start(COS[:], cosr)
            nc.scalar.dma_start(SIN[:], sinr)
            nc.gpsimd.dma_start(TRI2[:], tri2.rearrange("p (j m) -> p j m", j=2))
            nc.gpsimd.dma_start(IDENT[:], ident)
            nc.gpsimd.dma_start(SEL2[:], sel2)
            nc.gpsimd.dma_start(WO[:], wo.rearrange("(t p) e -> p t e", p=128))
            nc.vector.memset(VAUG[:], 1.0)

            # warm the PE (HAM un-throttle) during the startup DMA wait:
            # ~8us of dummy matmuls on a zeroed tile
            WARMT = cp.tile([128, SC], F16, tag="WARMT")
            nc.vector.memset(WARMT[:], 0.0)
            warm_ps = qps.tile([128, SC], F32, tag="q", name="warm")
            for _ in range(36):
                nc.tensor.matmul(warm_ps[:], WARMT[:, 0:128], WARMT[:],
                                 start=True, stop=True)

            # prefetch chunk 0's x tiles across three DMA queues
            xts_by_chunk = {}
            xts_by_chunk[0] = []
            for kt in range(KT):
                xt_t = xp.tile([128, SC], F16, tag="xt")
                eng = (nc.sync, nc.scalar, nc.gpsimd)[kt % 3]
                eng.dma_start(xt_t[:], xT3[kt, :, 0:SC])
                xts_by_chunk[0].append(xt_t)

            norm_pending = []

            def emit_norm(nqc, pr, ot0, ot1):
                nqsl = slice(nqc * SC, (nqc + 1) * SC)
                rec = npo.tile([33, SC], F32, tag="rec", name="rec")
                if nqc == 0 or n_cores == 1:
                    # rows 1:31 must be finite for the SEL2 matmul; the two
                    # pool buffers keep them at ~1.0 for later reuses
                    # (in-place recip of 1.0 stays 1.0)
                    nc.vector.memset(rec[:], 1.0)
                nc.vector.tensor_copy(rec[0:1, :], ot0[64:65, :])
                nc.vector.tensor_copy(rec[32:33, :], ot1[64:65, :])
                nc.vector.reciprocal_approx_fast(rec[:], rec[:])
                rbc_ps = bps.tile([128, 2, SC], F32, tag="sc", name="rbc")
                nc.tensor.matmul(
                    rbc_ps[:, 0, :], SEL2[0:33, :], rec[:], start=True, stop=True
                )
                rbc_sb = npo.tile([128, SC], F32, tag="rbc_sb", name="rbc_sb")
                nc.vector.tensor_copy(rbc_sb[:], rbc_ps[:, 0, :])
                att = (AT0, AT1)[pr]
                nc.vector.tensor_tensor(
                    att[0:64, nqsl], ot0[0:64, :], rbc_sb[0:64, :], OP.mult
                )
                nc.vector.tensor_tensor(
                    att[64:128, nqsl], ot1[0:64, :], rbc_sb[64:128, :], OP.mult
                )

            def emit_outproj(oqc):
                # output projection for chunk oqc's s-blocks
                for sb_i in range(4 * oqc, 4 * oqc + 4):
                    ssl = slice(sb_i * 128, (sb_i + 1) * 128)
                    for ec in range(DIM // 512):
                        esl = slice(ec * 512, (ec + 1) * 512)
                        o_ps = ops.tile(
                            [128, SC], F32, tag="ot", name=f"o_{sb_i}_{ec}"
                        )
                        for t in range(2):
                            att = (AT0, AT1)[t]
                            nc.tensor.matmul(
                                o_ps[:], att[:, ssl], WO[:, t, esl],
                                start=(t == 0), stop=(t == 1),
                            )
                        ost = op_pool.tile(
                            [128, 512], F16, tag="ost", name=f"os_{sb_i}_{ec}"
                        )
                        nc.any.tensor_copy(ost[:], o_ps[:])
                        nc.gpsimd.dma_start(out[ssl, esl], ost[:])

            for qc in range(NCH):
                sl = slice(qc * SC, (qc + 1) * SC)
                nkb = 4 * qc + 4
                qsl = sl

                # ---- projections for this chunk (two passes over 2 shared
                # psum slots so attention's 6 banks stay free) ----
                q0 = qps.tile([128, SC], F32, tag="q")
                q1 = qps.tile([128, SC], F32, tag="q")
                xts = xts_by_chunk.pop(qc)
                for kt in range(KT):
                    st, sp = kt == 0, kt == KT - 1
                    nc.tensor.matmul(
                        q0[:], WQ[:, kt, 0:128], xts[kt][:], start=st, stop=sp
                    )
                    nc.tensor.matmul(
                        q1[:], WQ[:, kt, 128:256], xts[kt][:], start=st, stop=sp
                    )
                # kv pass (reuses the resident xt tiles)
                kv = qps.tile([128, SC], F32, tag="q")
                for kt in range(KT):
                    nc.tensor.matmul(
                        kv[:], WKV[:, kt, :], xts[kt][:],
                        start=(kt == 0), stop=(kt == KT - 1),
                    )
                # prefetch next chunk's x tiles (xt pool holds 2 chunks)
                if qc + 1 < NCH:
                    nxt = []
                    for kt in range(KT):
                        xt_t = xp.tile([128, SC], F16, tag="xt")
                        nc.sync.dma_start(
                            xt_t[:], xT3[kt, :, (qc + 1) * SC : (qc + 2) * SC]
                        )
                        nxt.append(xt_t)
                    xts_by_chunk[qc + 1] = nxt

                # previous chunk's pair-1 normalization, overlapped with
                # this chunk's projection matmuls
                if norm_pending:
                    emit_norm(*norm_pending.pop())

                # rope q: fp16 copies via ACT, 2x-mode DVE ops writing the
                # per-pair interleaved REIM tiles directly
                q0s = rt.tile([128, SC], F16, tag="q0s")
                q1s = rt.tile([128, SC], F16, tag="q1s")
                nc.vector.tensor_copy(q0s[:], q0[:])
                nc.vector.tensor_copy(q1s[:], q1[:])
                t1 = rt.tile([128, SC], F16, tag="t1")
                t2 = rt.tile([128, SC], F16, tag="t2")
                t3 = rt.tile([128, SC], F16, tag="t3")
                t4 = rt.tile([128, SC], F16, tag="t4")
                nc.vector.tensor_tensor(t1[:], q0s[:], COS[:, sl], OP.mult)
                nc.vector.tensor_tensor(t2[:], q1s[:], SIN[:, sl], OP.mult)
                nc.vector.tensor_tensor(t3[:], q0s[:], SIN[:, sl], OP.mult)
                nc.vector.tensor_tensor(t4[:], q1s[:], COS[:, sl], OP.mult)
                for t, RT_ in enumerate((REIM0, REIM1)):
                    for half in range(2):
                        h = 2 * t + half
                        rq = slice(32 * h, 32 * h + 32)
                        nc.vector.tensor_tensor(
                            RT_[64 * half : 64 * half + 32, sl],
                            t1[rq, :], t2[rq, :], OP.subtract,
                        )
                        nc.vector.tensor_tensor(
                            RT_[64 * half + 32 : 64 * half + 64, sl],
                            t3[rq, :], t4[rq, :], OP.add,
                        )
                # kv rope
                kvs = rt.tile([128, SC], F16, tag="kvs")
                nc.vector.tensor_copy(kvs[:], kv[:])
                u1 = rt.tile([32, SC], F16, tag="u1")
                u2 = rt.tile([32, SC], F16, tag="u2")
                nc.vector.tensor_tensor(u1[:], kvs[0:32, :], COS[0:32, sl], OP.mult)
                nc.vector.tensor_tensor(u2[:], kvs[32:64, :], SIN[32:64, sl], OP.mult)
                nc.vector.tensor_tensor(KA4[0:32, sl], u1[:], u2[:], OP.subtract)
                u3 = rt.tile([32, SC], F16, tag="u3")
                u4 = rt.tile([32, SC], F16, tag="u4")
                nc.vector.tensor_tensor(u3[:], kvs[0:32, :], SIN[0:32, sl], OP.mult)
                nc.vector.tensor_tensor(u4[:], kvs[32:64, :], COS[32:64, sl], OP.mult)
                nc.vector.tensor_tensor(KA4[32:64, sl], u3[:], u4[:], OP.add)
                nc.sync.dma_start(KA4[64:128, sl], KA4[0:64, sl])
                for kb in range(4 * qc, 4 * qc + 4):
                    vp = ops.tile([128, 64], F16, tag="ot", name="vp")
                    nc.tensor.transpose(
                        vp[:], kvs[64:128, (kb % 4) * 128 : (kb % 4 + 1) * 128],
                        IDENT[64:128, 64:128],
                    )
                    nc.vector.tensor_copy(VAUG[:, kb, 0:HD], vp[:])

                # ---- attention for this chunk ----
                presc = []
                pend0 = [None]
                for pr in range(2):  # head pairs (0,1) and (2,3)
                    RT_ = (REIM0, REIM1)[pr]
                    # PV accumulators allocated lazily at first use so the
                    # ot-pool cycling orders them AFTER the previous pair's
                    # norm reads and the deferred out-proj tiles
                    ots = {}

                    def get_ots(ots=ots):
                        if not ots:
                            ots[0] = ops.tile([128, SC], F32, tag="ot", name="ot0")
                            ots[1] = ops.tile([128, SC], F32, tag="ot", name="ot1")
                        return ots

                    staged = []
                    hook_kb = min(4, nkb - 1)
                    thr = 2 if pr == 0 else hook_kb + 1

                    def stage(fn, thr=thr):
                        staged.append(fn)
                        if len(staged) > thr:
                            staged.pop(0)()

                    for kb in range(nkb):
                        if pr == 1 and kb == hook_kb and pend0[0] is not None:
                            # pair 0's norm chain, overlapped with pair 1's
                            # first score matmuls; then the previous chunk's
                            # out-proj fills the pipeline drain
                            emit_norm(*pend0[0])
                            pend0[0] = None
                            if qc > 0:
                                emit_outproj(qc - 1)
                        ksl = slice(kb * 128, (kb + 1) * 128)
                        jj = kb - 4 * qc
                        off = 0 if jj < 0 else jj * 128  # skip fully-masked q
                        qns = slice(qc * SC + off, (qc + 1) * SC)
                        sc_ps = (
                            presc.pop()
                            if presc
                            else bps.tile([128, 2, SC], F32, tag="sc", name="scp")
                        )
                        for j in range(2):
                            rs = slice(64 * j, 64 * j + 64)
                            nc.tensor.matmul(
                                sc_ps[:, j, off:SC], KA4[rs, ksl], RT_[rs, qns],
                                start=True, stop=True,
                                tile_position=(64 * j, 0),
                            )
                        if jj < 0:
                            # full block: fp16 p, normal PV
                            p_sb = pp.tile([128, 2, SC], F16, tag="p", name="pf")
                            if kb % 3 == 2:
                                nc.vector.tensor_scalar(
                                    p_sb[:].bitcast(I16), sc_ps[:],
                                    EXP_A, EXP_B, OP.mult, OP.add,
                                )
                            else:
                                nc.scalar.activation(
                                    p_sb[:], sc_ps[:], AF.Exp, scale=0.125
                                )

                            def pv_full(kb=kb, p_sb=p_sb):
                                o = get_ots()
                                for j in range(2):
                                    nc.tensor.matmul(
                                        o[j][0:65, :], VAUG[:, kb, 0:65],
                                        p_sb[:, j, :],
                                        start=(kb == 0), stop=False,
                                    )
                            stage(pv_full)
                        else:
                            p_sb = pp.tile([128, 2, SC], F16, tag="p")
                            if kb % 3 == 2:
                                # DVE exp-by-bitcast offloads the scalar engine
                                nc.vector.tensor_scalar(
                                    p_sb[:, :, off:SC].bitcast(I16),
                                    sc_ps[:, :, off:SC],
                                    EXP_A, EXP_B, OP.mult, OP.add,
                                )
                            else:
                                nc.scalar.activation(
                                    p_sb[:, :, off:SC], sc_ps[:, :, off:SC],
                                    AF.Exp, scale=0.125,
                                )
                            # causal mask on the diagonal 128-wide subblock
                            nc.vector.tensor_tensor(
                                p_sb[:, :, off : off + 128],
                                p_sb[:, :, off : off + 128],
                                TRI2[:], OP.mult,
                            )

                            def pv_diag(kb=kb, p_sb=p_sb, off=off):
                                o = get_ots()
                                for j in range(2):
                                    nc.tensor.matmul(
                                        o[j][0:65, off:SC], VAUG[:, kb, 0:65],
                                        p_sb[:, j, off:SC],
                                        start=(kb == 0), stop=(kb == nkb - 1),
                                    )
                            stage(pv_diag)
                    for fn in staged:
                        fn()

                    if pr == 0:
                        pend0[0] = (qc, 0, ots[0], ots[1])
                    else:
                        norm_pending.append((qc, 1, ots[0], ots[1]))

                if qc == NCH - 1:
                    emit_norm(*norm_pending.pop())
                    emit_outproj(qc)

    nc.compile()
    return nc


# host-side column permutations: all rope-even dims first, then all odds
_PERM256 = np.array(
    [64 * h + 2 * i for h in range(4) for i in range(32)]
    + [64 * h + 2 * i + 1 for h in range(4) for i in range(32)]
)
_PERM64 = np.array([2 * i for i in range(32)] + [2 * i + 1 for i in range(32)])

_cache = {}


def make_in_maps(x, cos, sin, wq, wk, wv, wo, n_groups=4):
    S = x.shape[1]
    cos_r = np.ascontiguousarray(np.tile(cos.T, (4, 1)), dtype=np.float16)
    sin_r = np.ascontiguousarray(np.tile(sin.T, (4, 1)), dtype=np.float16)
    tri = np.triu(np.ones((128, 128), dtype=np.float16))
    tri2 = np.ascontiguousarray(np.concatenate([tri, tri], axis=1))
    ident = np.eye(128, dtype=np.float16)
    sel2 = np.zeros((33, 128), dtype=np.float32)
    sel2[0, 0:64] = 1.0
    sel2[32, 64:128] = 1.0
    xTs = [np.ascontiguousarray(x[b].T.astype(np.float16)) for b in range(x.shape[0])]
    in_maps = []
    for c in range(x.shape[0] * n_groups):
        b, g = divmod(c, n_groups)
        wq_c = np.ascontiguousarray(wq[:, 256 * g + _PERM256].astype(np.float16))
        wk_c = wk[:, 64 * g + _PERM64]
        wv_c = wv[:, 64 * g : 64 * (g + 1)]
        wkv_c = np.ascontiguousarray(
            np.concatenate([wk_c, wv_c], axis=1), dtype=np.float16
        )
        wo_c = np.ascontiguousarray(wo[256 * g : 256 * (g + 1), :].astype(np.float16))
        in_maps.append(
            {
                "xT": xTs[b],
                "wq": wq_c,
                "wkv": wkv_c,
                "wo": wo_c,
                "cosr": cos_r,
                "sinr": sin_r,
                "tri2": tri2,
                "ident": ident,
                "sel2": sel2,
            }
        )
    return in_maps


def kernel(x, cos, sin, mask, wq, wk, wv, wo):
    x = np.asarray(x, dtype=np.float32)
    cos = np.asarray(cos, dtype=np.float32)
    sin = np.asarray(sin, dtype=np.float32)
    wq = np.asarray(wq, dtype=np.float32)
    wk = np.asarray(wk, dtype=np.float32)
    wv = np.asarray(wv, dtype=np.float32)
    wo = np.asarray(wo, dtype=np.float32)

    if "nc" not in _cache:
        _cache["nc"] = build_nc(S=x.shape[1], n_cores=8)
    nc = _cache["nc"]
    in_maps = make_in_maps(x, cos, sin, wq, wk, wv, wo)
    res = run_bass_kernel_spmd(nc, in_maps, list(range(8)))
    _cache["last"] = res
    outs = [r["out"].astype(np.float32) for r in res.results]
    final = np.stack(
        [outs[0] + outs[1] + outs[2] + outs[3], outs[4] + outs[5] + outs[6] + outs[7]],
        axis=0,
    )
    return final.astype(np.float32)


# revision 50
# speedup vs baseline: 1.0369x; 1.0369x over previous
"""GQA attention (RoPE + causal softmax + out-proj) on 8 TRN2 cores.

Sharding: one core per (batch b, kv-head-group g): 2 batches x 4 kv groups = 8
cores. Each core computes its group's 4 query heads end to end, including the
partial output projection through its 256 rows of wo; the host sums the 4
fp16 partial projections per batch in fp32.

Per-core kernel layout (all "transposed domain": feature dims on partitions,
sequence on the free axis):
  qT [256, S] = wq_g^T x^T, computed as matmul(lhsT=wq_g, rhs=xT); wq columns
  are host-permuted so psum M-tile 0 holds all even (te) rope components
  (4 heads x 32) and M-tile 1 all odd (to). RoPE is then whole-tile vector
  ops against cos/sin tables replicated per head. Same for k (rows 0:64 of
  the packed kv projection; v = rows 64:128).
  scores_T [k 128, q W] per (head-pair, k-block, q-chunk): one row-packed
  concurrent matmul pair (K=64 each at tile_position (0,0)/(64,0)); for
  diagonal-band k-blocks the fully-masked leading q-subblocks are skipped
  (W = 512-128*jj). exp via ACT (scale=1/8); causality enforced by a
  triangular multiply on the 128-wide diagonal subblock of p after exp.
  PV: out_T [65, q W] accumulated over k-blocks: matmul(lhsT=v_aug[k,65],
  rhs=p[k,qW]); v_aug column 64 is ones, so row 64 accumulates the softmax
  denominator. Normalization: reciprocal_approx_fast of that row, broadcast
  to 128 partitions via a tiny K=2 matmul against a selection matrix.
  Out-proj: matmul(lhsT=attT[c, s-block], rhs=wo[c, e-chunk]) -> [2048,1024]
  fp16 partial, DMA'd out.
"""

import os
import sys
import types

import numpy as np


def _ensure_axon_hooks_shim():
    """The agent image's antenv package lacks the axon_hooks submodule that
    concourse's trace path imports; install a stub so trace requests degrade
    to no-trace instead of crashing (a real hook can be set into the stub)."""
    try:
        import antenv.axon_hooks  # noqa: F401

        return
    except ImportError:
        pass
    try:
        import antenv
    except ImportError:
        return
    mod = types.ModuleType("antenv.axon_hooks")
    mod._AXON_NTFF_PROFILE_HOOK = None

    def get_axon_ntff_profile_hook():
        return mod._AXON_NTFF_PROFILE_HOOK

    def set_axon_ntff_profile_hook(hook):
        mod._AXON_NTFF_PROFILE_HOOK = hook

    mod.get_axon_ntff_profile_hook = get_axon_ntff_profile_hook
    mod.set_axon_ntff_profile_hook = set_axon_ntff_profile_hook
    sys.modules["antenv.axon_hooks"] = mod
    antenv.axon_hooks = mod


_ensure_axon_hooks_shim()

import concourse.bass as bass
import concourse.bacc as bacc
import concourse.mybir as mybir
import concourse.tile as tile
from concourse.bass_utils import run_bass_kernel_spmd

F32 = mybir.dt.float32
F16 = mybir.dt.float16
I16 = mybir.dt.int16
AF = mybir.ActivationFunctionType
OP = mybir.AluOpType

# Schraudolph-style fp16 exp-by-bitcast constants: trunc(x*EXP_A + EXP_B)
# interpreted as fp16 bits approximates exp(0.125*x) with ~3% max element
# error that cancels in the softmax ratio (calibrated for DVE truncation).
EXP_A = 1477.3197 * 0.125  # 2^10/ln2 * score scale
EXP_B = 15316.0

B, DIM = 2, 1024
NH, NKV, HD = 16, 4, 64
GH = NH // NKV  # query heads per kv group = 4
S_FULL = 2048
SC = 512  # q chunk width


def build_nc(S=S_FULL, n_cores=8):
    NCH = S // SC
    NKB = S // 128
    KT = DIM // 128  # 8 k-tiles over the model dim

    nc = bacc.Bacc(
        "TRN2", target_bir_lowering=False, debug=False, num_devices=n_cores
    )
    xT = nc.dram_tensor("xT", [DIM, S], F16, kind="ExternalInput").ap()
    wq = nc.dram_tensor("wq", [DIM, 256], F16, kind="ExternalInput").ap()
    wkv = nc.dram_tensor("wkv", [DIM, 128], F16, kind="ExternalInput").ap()
    wo = nc.dram_tensor("wo", [256, DIM], F16, kind="ExternalInput").ap()
    cosr = nc.dram_tensor("cosr", [128, S], F16, kind="ExternalInput").ap()
    sinr = nc.dram_tensor("sinr", [128, S], F16, kind="ExternalInput").ap()
    tri2 = nc.dram_tensor("tri2", [128, 256], F16, kind="ExternalInput").ap()
    ident = nc.dram_tensor("ident", [128, 128], F16, kind="ExternalInput").ap()
    sel2 = nc.dram_tensor("sel2", [33, 128], F32, kind="ExternalInput").ap()
    out = nc.dram_tensor("out", [S, DIM], F16, kind="ExternalOutput").ap()

    xT3 = xT.rearrange("(k p) s -> k p s", p=128)

    with tile.TileContext(nc) as tc:
        with (
            tc.tile_pool(name="const", bufs=1) as cp,
            tc.tile_pool(name="qps", bufs=2, space="PSUM") as qps,
            tc.tile_pool(name="scps", bufs=2, space="PSUM") as bps,
            tc.tile_pool(name="otps", bufs=2, space="PSUM") as ops,
            tc.tile_pool(name="xt", bufs=16) as xp,
            tc.tile_pool(name="rt", bufs=3) as rt,
            tc.tile_pool(name="pp", bufs=6) as pp,
            tc.tile_pool(name="np_", bufs=2) as npo,
            tc.tile_pool(name="op", bufs=3) as op_pool,
        ):
            COS = cp.tile([128, S], F16, tag="COS")
            SIN = cp.tile([128, S], F16, tag="SIN")
            WQ = cp.tile([128, KT, 256], F16, tag="WQ")
            WKV = cp.tile([128, KT, 128], F16, tag="WKV")
            WO = cp.tile([128, 2, DIM], F16, tag="WO")
            TRI2 = cp.tile([128, 2, 128], F16, tag="TRI2")
            IDENT = cp.tile([128, 128], F16, tag="IDENT")
            SEL2 = cp.tile([33, 128], F32, tag="SEL2")
            SEL16 = cp.tile([33, 128], F16, tag="SEL16")
            KA4 = cp.tile([128, S], F16, tag="KA4")  # KAB: [KA;KB;KA;KB]
            REIM0 = cp.tile([128, S], F16, tag="REIM0")
            REIM1 = cp.tile([128, S], F16, tag="REIM1")
            VAUG = cp.tile([128, NKB, 128], F16, tag="VAUG")
            AT0 = cp.tile([128, S], F16, tag="AT0")
            AT1 = cp.tile([128, S], F16, tag="AT1")

            # weight/const loads, split across queues so the critical path
            # (WQ/WKV + first x chunk on the sync queue) isn't serialized
            # behind the rest.
            wq_r = wq.rearrange("(k p) m -> p k m", p=128)
            wkv_r = wkv.rearrange("(k p) m -> p k m", p=128)
            for kt in range(KT):
                eng = (nc.sync, nc.scalar, nc.gpsimd)[kt % 3]
                eng.dma_start(WQ[:, kt, :], wq_r[:, kt, :])
                eng2 = (nc.scalar, nc.gpsimd, nc.sync)[kt % 3]
                eng2.dma_start(WKV[:, kt, :], wkv_r[:, kt, :])
            nc.scalar.dma_start(COS[:], cosr)
            nc.scalar.dma_start(SIN[:], sinr)
            nc.gpsimd.dma_start(TRI2[:], tri2.rearrange("p (j m) -> p j m", j=2))
            nc.gpsimd.dma_start(IDENT[:], ident)
            nc.gpsimd.dma_start(SEL2[:], sel2)
            nc.gpsimd.dma_start(WO[:], wo.rearrange("(t p) e -> p t e", p=128))
            nc.scalar.copy(SEL16[:], SEL2[:])
            nc.vector.memset(VAUG[:], 1.0)

            # warm the PE (HAM un-throttle) during the startup DMA wait:
            # ~8us of dummy matmuls on a zeroed tile
            WARMT = cp.tile([128, SC], F16, tag="WARMT")
            nc.vector.memset(WARMT[:], 0.0)
            warm_ps = qps.tile([128, SC], F32, tag="q", name="warm")
            for _ in range(36):
                nc.tensor.matmul(warm_ps[:], WARMT[:, 0:128], WARMT[:],
                                 start=True, stop=True)

            # prefetch chunk 0's x tiles across three DMA queues
            xts_by_chunk = {}
            xts_by_chunk[0] = []
            for kt in range(KT):
                xt_t = xp.tile([128, SC], F16, tag="xt")
                eng = (nc.sync, nc.scalar, nc.gpsimd)[kt % 3]
                eng.dma_start(xt_t[:], xT3[kt, :, 0:SC])
                xts_by_chunk[0].append(xt_t)

            norm_pending = []

            def emit_norm(nqc, pr, ot0, ot1):
                nqsl = slice(nqc * SC, (nqc + 1) * SC)
                rec = npo.tile([33, SC], F32, tag="rec", name="rec")
                if nqc == 0 or n_cores == 1:
                    # rows 1:31 must be finite for the SEL2 matmul; the two
                    # pool buffers keep them at ~1.0 for later reuses
                    # (in-place recip of 1.0 stays 1.0)
                    nc.vector.memset(rec[:], 1.0)
                nc.vector.tensor_copy(rec[0:1, :], ot0[64:65, :])
                nc.vector.tensor_copy(rec[32:33, :], ot1[64:65, :])
                nc.vector.reciprocal_approx_fast(rec[:], rec[:])
                rec16 = npo.tile([33, SC], F16, tag="rec16", name="rec16")
                nc.scalar.copy(rec16[:], rec[:])
                rbc_ps = bps.tile([128, 2, SC], F32, tag="sc", name="rbc")
                nc.tensor.matmul(
                    rbc_ps[:, 0, :], SEL16[0:33, :], rec16[:], start=True, stop=True
                )
                rbc_sb = npo.tile([128, SC], F32, tag="rbc_sb", name="rbc_sb")
                nc.vector.tensor_copy(rbc_sb[:], rbc_ps[:, 0, :])
                att = (AT0, AT1)[pr]
                nc.vector.tensor_tensor(
                    att[0:64, nqsl], ot0[0:64, :], rbc_sb[0:64, :], OP.mult
                )
                nc.vector.tensor_tensor(
                    att[64:128, nqsl], ot1[0:64, :], rbc_sb[64:128, :], OP.mult
                )

            def emit_outproj(oqc):
                # output projection for chunk oqc's s-blocks
                for sb_i in range(4 * oqc, 4 * oqc + 4):
                    ssl = slice(sb_i * 128, (sb_i + 1) * 128)
                    for ec in range(DIM // 512):
                        esl = slice(ec * 512, (ec + 1) * 512)
                        o_ps = ops.tile(
                            [128, SC], F32, tag="ot", name=f"o_{sb_i}_{ec}"
                        )
                        for t in range(2):
                            att = (AT0, AT1)[t]
                            nc.tensor.matmul(
                                o_ps[:], att[:, ssl], WO[:, t, esl],
                                start=(t == 0), stop=(t == 1),
                            )
                        ost = op_pool.tile(
                            [128, 512], F16, tag="ost", name=f"os_{sb_i}_{ec}"
                        )
                        nc.any.tensor_copy(ost[:], o_ps[:])
                        nc.gpsimd.dma_start(out[ssl, esl], ost[:])

            for qc in range(NCH):
                sl = slice(qc * SC, (qc + 1) * SC)
                nkb = 4 * qc + 4
                qsl = sl

                # ---- projections for this chunk (two passes over 2 shared
                # psum slots so attention's 6 banks stay free) ----
                q0 = qps.tile([128, SC], F32, tag="q")
                q1 = qps.tile([128, SC], F32, tag="q")
                xts = xts_by_chunk.pop(qc)
                for kt in range(KT):
                    st, sp = kt == 0, kt == KT - 1
                    nc.tensor.matmul(
                        q0[:], WQ[:, kt, 0:128], xts[kt][:], start=st, stop=sp
                    )
                    nc.tensor.matmul(
                        q1[:], WQ[:, kt, 128:256], xts[kt][:], start=st, stop=sp
                    )
                # kv pass (reuses the resident xt tiles)
                kv = qps.tile([128, SC], F32, tag="q")
                for kt in range(KT):
                    nc.tensor.matmul(
                        kv[:], WKV[:, kt, :], xts[kt][:],
                        start=(kt == 0), stop=(kt == KT - 1),
                    )
                # prefetch next chunk's x tiles (xt pool holds 2 chunks)
                if qc + 1 < NCH:
                    nxt = []
                    for kt in range(KT):
                        xt_t = xp.tile([128, SC], F16, tag="xt")
                        nc.sync.dma_start(
                            xt_t[:], xT3[kt, :, (qc + 1) * SC : (qc + 2) * SC]
                        )
                        nxt.append(xt_t)
                    xts_by_chunk[qc + 1] = nxt

                # previous chunk's pair-1 normalization, overlapped with
                # this chunk's projection matmuls
                if norm_pending:
                    emit_norm(*norm_pending.pop())

                # rope q: fp16 copies via ACT, 2x-mode DVE ops writing the
                # per-pair interleaved REIM tiles directly
                q0s = rt.tile([128, SC], F16, tag="q0s")
                q1s = rt.tile([128, SC], F16, tag="q1s")
                nc.vector.tensor_copy(q0s[:], q0[:])
                nc.vector.tensor_copy(q1s[:], q1[:])
                t1 = rt.tile([128, SC], F16, tag="t1")
                t2 = rt.tile([128, SC], F16, tag="t2")
                t3 = rt.tile([128, SC], F16, tag="t3")
                t4 = rt.tile([128, SC], F16, tag="t4")
                nc.vector.tensor_tensor(t1[:], q0s[:], COS[:, sl], OP.mult)
                nc.vector.tensor_tensor(t2[:], q1s[:], SIN[:, sl], OP.mult)
                nc.vector.tensor_tensor(t3[:], q0s[:], SIN[:, sl], OP.mult)
                nc.vector.tensor_tensor(t4[:], q1s[:], COS[:, sl], OP.mult)
                for t, RT_ in enumerate((REIM0, REIM1)):
                    for half in range(2):
                        h = 2 * t + half
                        rq = slice(32 * h, 32 * h + 32)
                        nc.vector.tensor_tensor(
                            RT_[64 * half : 64 * half + 32, sl],
                            t1[rq, :], t2[rq, :], OP.subtract,
                        )
                        nc.vector.tensor_tensor(
                            RT_[64 * half + 32 : 64 * half + 64, sl],
                            t3[rq, :], t4[rq, :], OP.add,
                        )
                # kv rope
                kvs = rt.tile([128, SC], F16, tag="kvs")
                nc.vector.tensor_copy(kvs[:], kv[:])
                u1 = rt.tile([32, SC], F16, tag="u1")
                u2 = rt.tile([32, SC], F16, tag="u2")
                nc.vector.tensor_tensor(u1[:], kvs[0:32, :], COS[0:32, sl], OP.mult)
                nc.vector.tensor_tensor(u2[:], kvs[32:64, :], SIN[32:64, sl], OP.mult)
                nc.vector.tensor_tensor(KA4[0:32, sl], u1[:], u2[:], OP.subtract)
                u3 = rt.tile([32, SC], F16, tag="u3")
                u4 = rt.tile([32, SC], F16, tag="u4")
                nc.vector.tensor_tensor(u3[:], kvs[0:32, :], SIN[0:32, sl], OP.mult)
                nc.vector.tensor_tensor(u4[:], kvs[32:64, :], COS[32:64, sl], OP.mult)
                nc.vector.tensor_tensor(KA4[32:64, sl], u3[:], u4[:], OP.add)
                nc.sync.dma_start(KA4[64:128, sl], KA4[0:64, sl])
                for kb in range(4 * qc, 4 * qc + 4):
                    vp = ops.tile([128, 64], F16, tag="ot", name="vp")
                    nc.tensor.transpose(
                        vp[:], kvs[64:128, (kb % 4) * 128 : (kb % 4 + 1) * 128],
                        IDENT[64:128, 64:128],
                    )
                    nc.vector.tensor_copy(VAUG[:, kb, 0:HD], vp[:])

                # ---- attention for this chunk ----
                presc = []
                pend0 = [None]
                for pr in range(2):  # head pairs (0,1) and (2,3)
                    RT_ = (REIM0, REIM1)[pr]
                    # PV accumulators allocated lazily at first use so the
                    # ot-pool cycling orders them AFTER the previous pair's
                    # norm reads and the deferred out-proj tiles
                    ots = {}

                    def get_ots(ots=ots):
                        if not ots:
                            ots[0] = ops.tile([128, SC], F32, tag="ot", name="ot0")
                            ots[1] = ops.tile([128, SC], F32, tag="ot", name="ot1")
                        return ots

                    staged = []
                    hook_kb = min(4, nkb - 1)
                    thr = 2 if pr == 0 else hook_kb + 1

                    def stage(fn, thr=thr):
                        staged.append(fn)
                        if len(staged) > thr:
                            staged.pop(0)()

                    for kb in range(nkb):
                        if pr == 1 and kb == hook_kb and pend0[0] is not None:
                            # pair 0's norm chain, overlapped with pair 1's
                            # first score matmuls; then the previous chunk's
                            # out-proj fills the pipeline drain
                            emit_norm(*pend0[0])
                            pend0[0] = None
                            if qc > 0:
                                emit_outproj(qc - 1)
                        ksl = slice(kb * 128, (kb + 1) * 128)
                        jj = kb - 4 * qc
                        off = 0 if jj < 0 else jj * 128  # skip fully-masked q
                        qns = slice(qc * SC + off, (qc + 1) * SC)
                        sc_ps = (
                            presc.pop()
                            if presc
                            else bps.tile([128, 2, SC], F32, tag="sc", name="scp")
                        )
                        for j in range(2):
                            rs = slice(64 * j, 64 * j + 64)
                            nc.tensor.matmul(
                                sc_ps[:, j, off:SC], KA4[rs, ksl], RT_[rs, qns],
                                start=True, stop=True,
                                tile_position=(64 * j, 0),
                            )
                        if jj < 0:
                            # full block: fp16 p, normal PV
                            p_sb = pp.tile([128, 2, SC], F16, tag="p", name="pf")
                            if kb % 3 == 2:
                                nc.vector.tensor_scalar(
                                    p_sb[:].bitcast(I16), sc_ps[:],
                                    EXP_A, EXP_B, OP.mult, OP.add,
                                )
                            else:
                                nc.scalar.activation(
                                    p_sb[:], sc_ps[:], AF.Exp, scale=0.125
                                )

                            def pv_full(kb=kb, p_sb=p_sb):
                                o = get_ots()
                                for j in range(2):
                                    nc.tensor.matmul(
                                        o[j][0:65, :], VAUG[:, kb, 0:65],
                                        p_sb[:, j, :],
                                        start=(kb == 0), stop=False,
                                    )
                            stage(pv_full)
                        else:
                            p_sb = pp.tile([128, 2, SC], F16, tag="p")
                            if kb % 3 == 2:
                                # DVE exp-by-bitcast offloads the scalar engine
                                nc.vector.tensor_scalar(
                                    p_sb[:, :, off:SC].bitcast(I16),
                                    sc_ps[:, :, off:SC],
                                    EXP_A, EXP_B, OP.mult, OP.add,
                                )
                            else:
                                nc.scalar.activation(
                                    p_sb[:, :, off:SC], sc_ps[:, :, off:SC],
                                    AF.Exp, scale=0.125,
                                )
                            # causal mask on the diagonal 128-wide subblock
                            nc.vector.tensor_tensor(
                                p_sb[:, :, off : off + 128],
                                p_sb[:, :, off : off + 128],
                                TRI2[:], OP.mult,
                            )

                            def pv_diag(kb=kb, p_sb=p_sb, off=off):
                                o = get_ots()
                                for j in range(2):
                                    nc.tensor.matmul(
                                        o[j][0:65, off:SC], VAUG[:, kb, 0:65],
                                        p_sb[:, j, off:SC],
                                        start=(kb == 0), stop=(kb == nkb - 1),
                                    )
                            stage(pv_diag)
                    for fn in staged:
                        fn()

                    if pr == 0:
                        pend0[0] = (qc, 0, ots[0], ots[1])
                    else:
                        norm_pending.append((qc, 1, ots[0], ots[1]))

                if qc == NCH - 1:
                    emit_norm(*norm_pending.pop())
                    emit_outproj(qc)

    nc.compile()
    return nc


# host-side column permutations: all rope-even dims first, then all odds
_PERM256 = np.array(
    [64 * h + 2 * i for h in range(4) for i in range(32)]
    + [64 * h + 2 * i + 1 for h in range(4) for i in range(32)]
)
_PERM64 = np.array([2 * i for i in range(32)] + [2 * i + 1 for i in range(32)])

_cache = {}


def make_in_maps(x, cos, sin, wq, wk, wv, wo, n_groups=4):
    S = x.shape[1]
    cos_r = np.ascontiguousarray(np.tile(cos.T, (4, 1)), dtype=np.float16)
    sin_r = np.ascontiguousarray(np.tile(sin.T, (4, 1)), dtype=np.float16)
    tri = np.triu(np.ones((128, 128), dtype=np.float16))
    tri2 = np.ascontiguousarray(np.concatenate([tri, tri], axis=1))
    ident = np.eye(128, dtype=np.float16)
    sel2 = np.zeros((33, 128), dtype=np.float32)
    sel2[0, 0:64] = 1.0
    sel2[32, 64:128] = 1.0
    xTs = [np.ascontiguousarray(x[b].T.astype(np.float16)) for b in range(x.shape[0])]
    in_maps = []
    for c in range(x.shape[0] * n_groups):
        b, g = divmod(c, n_groups)
        wq_c = np.ascontiguousarray(wq[:, 256 * g + _PERM256].astype(np.float16))
        wk_c = wk[:, 64 * g + _PERM64]
        wv_c = wv[:, 64 * g : 64 * (g + 1)]
        wkv_c = np.ascontiguousarray(
            np.concatenate([wk_c, wv_c], axis=1), dtype=np.float16
        )
        wo_c = np.ascontiguousarray(wo[256 * g : 256 * (g + 1), :].astype(np.float16))
        in_maps.append(
            {
                "xT": xTs[b],
                "wq": wq_c,
                "wkv": wkv_c,
                "wo": wo_c,
                "cosr": cos_r,
                "sinr": sin_r,
                "tri2": tri2,
                "ident": ident,
                "sel2": sel2,
            }
        )
    return in_maps


def kernel(x, cos, sin, mask, wq, wk, wv, wo):
    x = np.asarray(x, dtype=np.float32)
    cos = np.asarray(cos, dtype=np.float32)
    sin = np.asarray(sin, dtype=np.float32)
    wq = np.asarray(wq, dtype=np.float32)
    wk = np.asarray(wk, dtype=np.float32)
    wv = np.asarray(wv, dtype=np.float32)
    wo = np.asarray(wo, dtype=np.float32)

    if "nc" not in _cache:
        _cache["nc"] = build_nc(S=x.shape[1], n_cores=8)
    nc = _cache["nc"]
    in_maps = make_in_maps(x, cos, sin, wq, wk, wv, wo)
    res = run_bass_kernel_spmd(nc, in_maps, list(range(8)))
    _cache["last"] = res
    outs = [r["out"].astype(np.float32) for r in res.results]
    final = np.stack(
        [outs[0] + outs[1] + outs[2] + outs[3], outs[4] + outs[5] + outs[6] + outs[7]],
        axis=0,
    )
    return final.astype(np.float32)
